# revision 51
# baseline (speedup 1.0000x reference)
"""TRN2 Bass kernel for nn_BiAttention (B=48, S=512, H=768) on 8 NeuronCores.

Data-parallel: 6 samples per core, weights replicated.

Per-sample math (matches the reference exactly):
  Q = x @ Wq.T + bq ; K = x @ Wk.T + bk ; V = x @ Wv.T + bv
  scores = Q @ K.T / sqrt(H) + A        (A = segment allow/additive mask)
  attn = softmax(scores, axis=-1)
  out = tanh((attn @ V) @ W0.T + b0) @ W1.T + b1 + x

Kernel design (fused form):
  - Q.K fusion: scores = x @ Bsc @ x.T + u(k) + v(q) + c + A, with
    Bsc = Wq.T@Wk/sqrt(H) precomputed on the host, and the bias
    cross-terms u = x@(Wk.T bq)/sqrt(H), v = x@(Wq.T bk)/sqrt(H),
    c = bq.bk/sqrt(H) computed per-sample on the host. Only ONE device
    projection (P1T = Bsc.T @ xT) instead of Q and K.
  - V.W0 fusion: (attn@V)@W0.T + b0 = attn@(x@Wc.T) + b0' with
    Wc = W0@Wv and b0' = b0 + W0@bv (uses sum(attn_row)=1). FC0
    disappears; tanh applies directly to the PV psum.
  - scores computed TRANSPOSED (sT[k,q]) so the attention matrix is
    already in the right layout for the P@V matmul — no transposes.
  - The additive mask + u/v/c terms are rank-4: host packs l4=[rowQ*64,
    rowC*64, u+c, 1] (k-side) and r4=[isq, 1-isq, 1, v] (q-side); one
    K=4 matmul accumulates all of it into the scores psum.
  - MAJORITY-FIRST permutation (host): per sample, the first 256
    positions hold 256 tokens of the larger segment (always >= 256
    tokens). All pairs inside [0,256) are then same-segment ->
    disallowed, so the (k<256, q<256) quadrant of scoresT is NEVER
    computed: stage C k-tiles 0-1 only produce q in [256,512), the PV
    k-accumulation for q<256 skips k-tiles 0-1, and zsum/exp/normalize
    shrink accordingly (~25% off all attention-stage work). The host
    un-permutes the output.
  - softmax without max-subtraction (scores are O(1); -1e9 -> exp = 0;
    P1T is stored x64 so exp uses scale=1/64). Column sums via a
    ones^T DoubleRow matmul; 1/Z broadcast back with a K=1 matmul;
    expT normalized in place.
  - All heavy matmuls are fp8e4m3 DoubleRow (2 weights/PE cell):
    P1T/Vc/FC1 contract h-pairs, scores contract o-pairs, PV contracts
    k-pairs. Host prescales: Bsc x8192, Wc x256, W1 x256 (fp8e4m3
    subnormal floor is ~2^-9), descaled in psum->sbuf copies / on host.
  - b1 + x + the FC1 1/256 descale are applied on the HOST in fp32;
    the device ships the raw FC1 psum.
  - GPSIMD/Pool cannot access PSUM, so all psum->sbuf copies are
    balanced across DVE and ACT by a cost-model-aware router; Pool
    takes the SBUF-only softmax-normalize mul for the k-tiles 0-1
    range while DVE (right behind the reciprocal in its own queue)
    normalizes the k-tiles 2-3 range the next PV stage needs first.
  - PSUM (8 banks): P1T/FC1/scores pair-groups rotate on a 2-buf
    2-bank pool; Vc/PV and the last sample's FC1 singles on a 3-buf
    1-bank pool; zsum and the scores-half group share the last bank.
  - All loads ride ONE HWDGE queue in priority order (x0, Bsc, WcT,
    l4/r4, x1, W1T, b0p, x2-5) as whole contiguous transfers: the DMA
    unit is serial, strided transfers cost ~2x per byte, and a load on
    any second queue can cut the line ahead of a compute-critical one.
  - PE p-state warmup: dummy matmuls from ~0.6us start the tensor
    engine's 3us ramp clock during the initial DMA wait.
  - Software pipeline is depth ~3.5: iteration i runs projections of
    sample i, attention of sample i-1, PV+tanh of sample i-2 and FC1+
    store of sample i-3 (FC1 shifted one iteration so the tanh chain
    never stalls PE); the last sample's PV and FC1 are fast-tracked to
    compress the pipeline tail.
"""

import numpy as np
import ml_dtypes

B, S, H = 48, 512, 768
NCORES = 8
BPC = B // NCORES  # samples per core
P = 128
HC = H // P   # 6 chunks of 128 over hidden dim
HJ = HC // 2  # 3 DoubleRow pair-groups over hidden dim
SC = S // P   # 4 chunks of 128 over sequence dim
SJ = SC // 2  # 2 DoubleRow pair-groups over sequence dim
SH = S // 2   # majority-pure boundary (positions [0,SH) are one segment)
NEG = -16384.0  # e5m2-exact; exp((s-16384)/64) underflows to exactly 0
RS = float(1.0 / np.sqrt(np.float32(H)))
WS_B = 8192.0   # Bsc prescale
SB_OUT = 64.0   # P1T storage scale (exp compensates with scale=1/64)
WS_C = 256.0    # Wc prescale
WS_1 = 256.0    # W1 prescale
WARMUP_MM = 17  # dummy matmuls starting the PE p-state ramp clock

_cache = {}
_STAGE_MARKS = []  # (inst_creation_watermark, label) for attribution


def _build_program():
    import concourse.bass as bass
    import concourse.mybir as mybir
    import concourse.tile as tile
    from concourse import bacc

    f32 = mybir.dt.float32
    bf16 = mybir.dt.bfloat16
    f8 = mybir.dt.float8e4
    f85 = mybir.dt.float8e5
    AF = mybir.ActivationFunctionType
    ALU = mybir.AluOpType
    DR = mybir.MatmulPerfMode.DoubleRow

    nc = bacc.Bacc("TRN2", target_bir_lowering=False, debug=False)

    # ---- DRAM tensors (per-core) ----
    xT_d = nc.dram_tensor("xT", [BPC, H, S], f8, kind="ExternalInput")
    w_d = {
        name: nc.dram_tensor(name, [H, H], f8, kind="ExternalInput")
        for name in ["Bsc", "WcT", "W1T"]
    }
    b0p_d = nc.dram_tensor("b0p", [H], f32, kind="ExternalInput")
    # [2, BPC, 2, S]: 2 partitions x (sample, free-dim DoubleRow pair, S)
    l4_d = nc.dram_tensor("l4", [2, BPC, 2, S], f85, kind="ExternalInput")
    r4_d = nc.dram_tensor("r4", [2, BPC, 2, S], f85, kind="ExternalInput")
    outT_d = nc.dram_tensor("outT", [BPC, H, S], bf16, kind="ExternalOutput")

    with tile.TileContext(nc) as tc:
        with (
            tc.tile_pool(name="wpool", bufs=1) as wpool,
            tc.tile_pool(name="xpool", bufs=1) as xpool,
            tc.tile_pool(name="ppool", bufs=2) as ppool,
            tc.tile_pool(name="vpool", bufs=3) as vpool,
            tc.tile_pool(name="epool", bufs=2) as epool,
            tc.tile_pool(name="rpool", bufs=2) as rpool,
            tc.tile_pool(name="opool", bufs=2) as opool,
            tc.tile_pool(name="zpool", bufs=2) as zpool,
            tc.tile_pool(name="psum2", bufs=2, space="PSUM") as psum2,
            tc.tile_pool(name="psum1", bufs=3, space="PSUM") as psum1,
            tc.tile_pool(name="psumZ", bufs=1, space="PSUM") as psumZ,
        ):
            # --- three-engine copy balancer: psum -> sbuf
            # (out = in*scale [+ bias]). Routes each flexible op to the
            # engine minimizing (backlog + op cost) using the cost-model
            # rates: DVE 1.04 ns/col, ACT 0.83, Pool 1.39 (copies) /
            # 1.98 (tensor_tensor), plus fixed per-op overheads.
            eng_state = {"dve": 0.0, "act": 0.0, "pool": 200.0}
            # NOTE: GPSIMD/Pool cannot access PSUM (BIR verifier) — all
            # psum->sbuf copies go to DVE/ACT; Pool gets SBUF-only work
            # (the softmax normalize muls).
            RATE = {"dve": 1.042, "act": 0.833, "pool": 1.389}
            FIXC = {"dve": 130.0, "act": 190.0, "pool": 100.0}

            def op_cost(eng, cols, rate_scale=1.0):
                r = RATE[eng] * (rate_scale if eng == "pool" else 1.0)
                return cols * r + FIXC[eng]

            def pick(cols, allowed=("dve", "act"), rate_scale=1.0):
                return min(
                    allowed,
                    key=lambda e: eng_state[e] + op_cost(e, cols, rate_scale),
                )

            def copy_out(dst, src, scale=None, bias=None, force=None,
                         allowed=("dve", "act")):
                cols = 1
                for d in dst.shape[1:]:
                    cols *= d
                eng = force if force is not None else pick(cols, allowed)
                eng_state[eng] += op_cost(eng, cols)
                if eng == "dve":
                    if bias is None and scale is None:
                        nc.vector.tensor_copy(dst, src)
                    elif bias is None:
                        nc.vector.tensor_scalar_mul(dst, src, scale)
                    elif scale is None:
                        nc.vector.tensor_scalar(
                            dst, src, scalar1=bias, scalar2=None, op0=ALU.add
                        )
                    else:
                        nc.vector.tensor_scalar(
                            dst, src, scalar1=scale, scalar2=bias,
                            op0=ALU.mult, op1=ALU.add,
                        )
                elif eng == "act":
                    nc.scalar.activation(
                        dst, src, func=AF.Identity,
                        bias=0.0 if bias is None else bias,
                        scale=1.0 if scale is None else scale,
                    )
                else:
                    assert bias is None
                    if scale is None:
                        nc.gpsimd.tensor_copy(dst, src)
                    else:
                        nc.gpsimd.tensor_scalar_mul(dst, src, scale)

            def mul_out(dst, a, bb, force=None):
                cols = 1
                for d in dst.shape[1:]:
                    cols *= d
                eng = force if force is not None else "pool"
                
                eng_state[eng] += op_cost(eng, cols, rate_scale=1.43)
                if eng == "dve":
                    nc.vector.tensor_mul(dst, a, bb)
                else:
                    nc.gpsimd.tensor_mul(dst, a, bb)

            w_sb = {}

            # warmup + zsum constants FIRST on the Pool queue (it is idle
            # for the first ~0.5us) so the PE ramp clock starts ASAP.
            ones_k = wpool.tile([P, 2, P], f8, tag="ones_k")
            nc.gpsimd.memset(ones_k, 1.0)

            # ---- startup DMA plan ----
            # sync/HWDGE queue: x0+Bsc pair-interleaved (the first matmul
            # wave needs only pair 0 of each), then x1..x5.
            # gpsimd/SWDGE (desc-gen on the Pool engine, transfers yield
            # to the critical x0/Bsc stream): l4, r4, WcT, W1T, b0p.
            x_t = {}
            for b in range(BPC):
                x_t[b] = xpool.tile([P, HC, S], f8, tag=f"xT{b}",
                                    name=f"xT{b}")
            bsc_t = wpool.tile([P, HC, H], f8, tag="Bsc")
            bsc_r = w_d["Bsc"].ap().rearrange("(c p) o -> p c o", p=P)
            w_sb["Bsc"] = bsc_t
            # x0 and the weights as whole contiguous transfers — strided
            # (row/column-sliced) DMAs cost ~2x per byte, so whole tensors
            # in strict priority order beat any finer interleave.
            x0r = xT_d.ap()[0].rearrange("(c p) s -> p c s", p=P)
            nc.sync.dma_start(x_t[0][:], x0r)
            nc.sync.dma_start(bsc_t[:], bsc_r)

            # ALL loads ride the single HWDGE queue in priority order —
            # the DMA unit transfers strictly in entry order, so any load
            # on a second queue can cut the line in front of a
            # compute-critical one. Priority: x0, Bsc (A of s0), WcT
            # halves (B of s0), l4/r4 (C of s0), x1, W1T+b0p (F/G), x2-5.
            t = wpool.tile([P, HC, H], f8, tag="WcT")
            wcr = w_d["WcT"].ap().rearrange("(c p) o -> p c o", p=P)
            nc.sync.dma_start(t[:], wcr)
            w_sb["WcT"] = t

            m_l4 = wpool.tile([2, BPC, 2, S], f85, tag="l4")
            nc.sync.dma_start(m_l4[:], l4_d.ap())
            m_r4 = wpool.tile([2, BPC, 2, S], f85, tag="r4")
            nc.sync.dma_start(m_r4[:], r4_d.ap())
            nc.sync.dma_start(
                x_t[1][:], xT_d.ap()[1].rearrange("(c p) s -> p c s", p=P)
            )
            t = wpool.tile([P, HC, H], f8, tag="W1T")
            nc.sync.dma_start(
                t[:], w_d["W1T"].ap().rearrange("(c p) o -> p c o", p=P)
            )
            w_sb["W1T"] = t
            b0p_sb = wpool.tile([P, HC], f32, tag="b0p")
            nc.sync.dma_start(
                b0p_sb[:], b0p_d.ap().rearrange("(c p) -> p c", p=P)
            )
            for b in range(2, BPC):
                nc.sync.dma_start(
                    x_t[b][:], xT_d.ap()[b].rearrange("(c p) s -> p c s", p=P)
                )

            # ---- PE p-state warmup: the ramp clock starts at the first
            # matmul, so a burst of early dummy matmuls (values
            # irrelevant) bridges the initial DMA wait at ramp speed and
            # hands off to the first real matmul with the clock warm.
            wu_r = wpool.tile([P, 2, S], f8, tag="wu_r")
            nc.gpsimd.memset(wu_r, 0.0)
            wu_ps = psum1.tile([P, S], f32, tag="ps1")
            for k in range(15):
                nc.tensor.matmul(
                    wu_ps[:, 0:P], lhsT=ones_k[:], rhs=ones_k[:],
                    start=True, stop=True, perf_mode=DR,
                )
            for k in range(WARMUP_MM):
                nc.tensor.matmul(
                    wu_ps[:], lhsT=ones_k[:], rhs=wu_r[:],
                    start=True, stop=True, perf_mode=DR,
                )
            wu_sink = zpool.tile([P, 8], f32, tag="wu_sink")
            nc.vector.tensor_copy(wu_sink[:], wu_ps[:, 0:8])

            def stage_a(b):
                """P1T[h', q] = (Bsc.T @ xT), stored fp8 at x64 scale:
                3 pair thunks (2-bank psum, one 1024-col copy each)."""
                xt = x_t[b]
                p1 = ppool.tile([P, HC, S], f8, tag="P1T")

                def group(jo):
                    ps = psum2.tile([P, 2, S], f32, tag="ps2")
                    for i in range(2):
                        o = 2 * jo + i
                        for j in range(HJ):
                            nc.tensor.matmul(
                                ps[:, i, :],
                                lhsT=w_sb["Bsc"][:, 2 * j:2 * j + 2,
                                                 o * P:(o + 1) * P],
                                rhs=xt[:, 2 * j:2 * j + 2, :],
                                start=(j == 0), stop=(j == HJ - 1),
                                perf_mode=DR,
                            )
                    copy_out(p1[:, 2 * jo:2 * jo + 2, :], ps[:],
                             scale=float(SB_OUT / WS_B))

                return p1, [lambda jo=jo: group(jo) for jo in range(HJ)]

            def stage_b(b):
                """Vc[s, o] = x @ Wc.T (PV's lhsT layout): 8 half thunks"""
                xt = x_t[b]
                vc = vpool.tile([P, SC, H], f8, tag="Vc")
                HH = H // 2

                def group(g):
                    s4, half = divmod(g, 2)
                    ps = psum1.tile([P, S], f32, tag="ps1")
                    for j in range(HJ):
                        nc.tensor.matmul(
                            ps[:, :HH],
                            lhsT=xt[:, 2 * j:2 * j + 2, s4 * P:(s4 + 1) * P],
                            rhs=w_sb["WcT"][:, 2 * j:2 * j + 2,
                                            half * HH:(half + 1) * HH],
                            start=(j == 0), stop=(j == HJ - 1),
                            perf_mode=DR,
                        )
                    copy_out(vc[:, s4, half * HH:(half + 1) * HH],
                             ps[:, :HH], scale=float(1.0 / WS_C))

                return vc, [lambda g=g: group(g) for g in range(2 * SC)]

            def stage_c(b, p1):
                """scoresT[k,q]*64 = x.T @ P1T + l4.T @ r4 ; exp(/64).
                k-tiles 0-1 (the majority-pure block) only produce
                q in [SH,S) — the (k<SH, q<SH) quadrant is all
                same-segment -> exp == 0 -> never computed.
                3 thunks: [k-tiles 0-1 | q>=SH] (1 bank), k-tile 2, 3."""
                xt = x_t[b]
                et = epool.tile([P, SC, S], f8, tag="expT")

                def group_h():
                    psf = psumZ.tile([P, S], f32, tag="psz")
                    ps = psf.rearrange("p (i n) -> p i n", i=2)
                    for i in range(2):
                        nc.tensor.matmul(
                            ps[:, i, :],
                            lhsT=m_l4[:, b, :, i * P:(i + 1) * P],
                            rhs=m_r4[:, b, :, SH:],
                            start=True, stop=False,
                            perf_mode=DR,
                        )
                        for j in range(HJ):
                            nc.tensor.matmul(
                                ps[:, i, :],
                                lhsT=xt[:, 2 * j:2 * j + 2, i * P:(i + 1) * P],
                                rhs=p1[:, 2 * j:2 * j + 2, SH:],
                                start=False, stop=(j == HJ - 1),
                                perf_mode=DR,
                            )
                    eng_state["act"] += op_cost("act", 2 * (S - SH))
                    nc.scalar.activation(
                        et[:, 0:2, SH:], ps[:],
                        func=AF.Exp, scale=float(1.0 / SB_OUT),
                    )

                def group_f():
                    ps = psum2.tile([P, 2, S], f32, tag="ps2")
                    for i in range(2):
                        k4 = 2 + i
                        nc.tensor.matmul(
                            ps[:, i, :],
                            lhsT=m_l4[:, b, :, k4 * P:(k4 + 1) * P],
                            rhs=m_r4[:, b, :, :],
                            start=True, stop=False,
                            perf_mode=DR,
                        )
                        for j in range(HJ):
                            nc.tensor.matmul(
                                ps[:, i, :],
                                lhsT=xt[:, 2 * j:2 * j + 2,
                                        k4 * P:(k4 + 1) * P],
                                rhs=p1[:, 2 * j:2 * j + 2, :],
                                start=False, stop=(j == HJ - 1),
                                perf_mode=DR,
                            )
                    eng_state["act"] += op_cost("act", 2 * S)
                    nc.scalar.activation(
                        et[:, 2:4, :], ps[:],
                        func=AF.Exp, scale=float(1.0 / SB_OUT),
                    )

                return et, [group_h, group_f, None]

            def zsum(et):
                """column sums of expT, broadcast to all 128 partitions.
                Full-width matmul over k-tiles 2-3 first (start=True),
                then the majority-pure k-tiles 0-1 accumulate only into
                q in [SH,S)."""
                ps_z = psumZ.tile([P, S], f32, tag="psz")
                nc.tensor.matmul(
                    ps_z[:],
                    lhsT=ones_k[:],
                    rhs=et[:, 2:4, :],
                    start=True, stop=False,
                    perf_mode=DR,
                )
                nc.tensor.matmul(
                    ps_z[:, SH:],
                    lhsT=ones_k[:],
                    rhs=et[:, 0:2, SH:],
                    start=False, stop=True,
                    perf_mode=DR,
                )
                return ps_z

            def zb_norm(et, ps_z):
                """reciprocal of broadcast sums, normalize expT in place.
                k-tiles 2-3 (needed first by PV) normalized first."""
                rz = zpool.tile([P, S], bf16, tag="rz")
                eng_state["dve"] += op_cost("dve", S)
                with nc.allow_low_precision(reason="1/Z in bf16; expT fp8"):
                    nc.vector.reciprocal(rz[:], ps_z[:])
                zb_b = rz[:, None, :].to_broadcast((P, 2, S))
                mul_out(et[:, 2:4, :], et[:, 2:4, :], zb_b, force="dve")
                zb_h = rz[:, None, SH:].to_broadcast((P, 2, S - SH))
                mul_out(et[:, 0:2, SH:], et[:, 0:2, SH:], zb_h, force="pool")

            def stage_f(b, vc, et):
                """PV + tanh for sample b -> hT (fp8): 6 single thunks.
                k-tiles 2-3 full width first (start=True), then k-tiles
                0-1 accumulate only q in [SH,S)."""
                ht = rpool.tile([P, HC, S], f8, tag="hT")

                def group(h):
                    ps = psum1.tile([P, S], f32, tag="ps1")
                    nc.tensor.matmul(
                        ps[:],
                        lhsT=vc[:, 2:4, h * P:(h + 1) * P],
                        rhs=et[:, 2:4, :],
                        start=True, stop=False,
                        perf_mode=DR,
                    )
                    nc.tensor.matmul(
                        ps[:, SH:],
                        lhsT=vc[:, 0:2, h * P:(h + 1) * P],
                        rhs=et[:, 0:2, SH:],
                        start=False, stop=True,
                        perf_mode=DR,
                    )
                    eng_state["act"] += op_cost("act", S)
                    nc.scalar.activation(
                        ht[:, h, :], ps[:], func=AF.Tanh,
                        bias=b0p_sb[:, h:h + 1],
                    )

                return ht, [lambda h=h: group(h) for h in range(HC)]

            def stage_g(b, ht, final=False, force_list=None):
                """FC1 (raw psum, x256) + store; host adds b1+x and /256.
                3 pair thunks (one 1024-col copy + pair store each).
                final=True splits each copy across DVE+ACT so the
                kernel-tail copy+store chain is as short as possible."""
                ot = opool.tile([P, HC, S], bf16, tag="outT")
                our = outT_d.ap()[b].rearrange("(c p) s -> p c s", p=P)

                def group(jo):
                    ps = psum2.tile([P, 2, S], f32, tag="ps2")
                    for i in range(2):
                        o = 2 * jo + i
                        for j in range(HJ):
                            nc.tensor.matmul(
                                ps[:, i, :],
                                lhsT=w_sb["W1T"][:, 2 * j:2 * j + 2,
                                                 o * P:(o + 1) * P],
                                rhs=ht[:, 2 * j:2 * j + 2, :],
                                start=(j == 0), stop=(j == HJ - 1),
                                perf_mode=DR,
                            )
                    copy_out(ot[:, 2 * jo:2 * jo + 2, :], ps[:])
                    nc.sync.dma_start(
                        our[:, 2 * jo:2 * jo + 2, :],
                        ot[:, 2 * jo:2 * jo + 2, :],
                    )

                def group_final(o):
                    # single-bank psum on the roomy ps1 pool; copies
                    # alternate DVE/ACT so the kernel-tail chain is short
                    ps = psum1.tile([P, S], f32, tag="ps1")
                    for j in range(HJ):
                        nc.tensor.matmul(
                            ps[:],
                            lhsT=w_sb["W1T"][:, 2 * j:2 * j + 2,
                                             o * P:(o + 1) * P],
                            rhs=ht[:, 2 * j:2 * j + 2, :],
                            start=(j == 0), stop=(j == HJ - 1),
                            perf_mode=DR,
                        )
                    copy_out(ot[:, o, :], ps[:],
                             force=["dve", "act"][o % 2])
                    if o % 2 == 1:
                        nc.sync.dma_start(
                            our[:, o - 1:o + 1, :],
                            ot[:, o - 1:o + 1, :],
                        )

                if final:
                    return [lambda o=o: group_final(o) for o in range(HC)]
                return [lambda jo=jo: group(jo) for jo in range(HJ)]

            # Depth-3.5 software pipeline: iteration i emits sample i's
            # projections (A=P1T, B=Vc), sample i-1's attention (C=scores
            # +exp, D=zsum, E=norm), sample i-2's PV+tanh (F) and sample
            # i-3's FC1+store (G), interleaved so every cross-engine chain
            # (exp->zsum, recip->norm->PV, tanh->FC1) has microseconds of
            # independent PE work queued behind it.
            state = {}   # sample index -> dict of live tiles/thunks

            def emit(th, label=None):
                if th is not None:
                    if label is not None:
                        _STAGE_MARKS.append((len(nc.inst_map), label))
                    th()

            NA, NB, NF, NG = HJ, 2 * SC, HC, HJ
            for i in range(BPC + 2):
                cur = None
                if i < BPC:
                    cur = {"b": i}
                    p1, cur["A"] = stage_a(i)
                    vc, cur["B"] = stage_b(i)
                    cur["p1"], cur["vc"] = p1, vc
                mid = state.get(i - 1)   # sample doing attention this round
                if mid is not None:
                    et, mid["C"] = stage_c(mid["b"], mid["p1"])
                    mid["et"] = et
                pv = state.get(i - 2)    # sample doing PV+tanh this round
                if pv is not None and "ht" in pv:
                    pv = None            # already fast-tracked (last sample)
                old = state.get(i - 3)   # sample storing this round

                A = cur["A"] if cur else [None] * NA
                Bg = cur["B"] if cur else [None] * NB
                Cg = mid["C"] if mid else [None] * 3
                G = (stage_g(old["b"], old["ht"],
                             force_list=(["act", "act", "act"]
                                         if i == BPC else None))
                     if old is not None else [None] * NG)

                if i == BPC:
                    # penultimate iteration: the last sample's attention
                    # chain gets priority — its recip (DVE) and norm
                    # (Pool) must land on clear queues, so G(s3) copies
                    # are forced onto ACT behind the exps.
                    emit(Cg[0], "C0")
                    emit(Cg[1], "C1")
                    ht, fthunks = stage_f(pv["b"], pv["vc"], pv["et"])
                    pv["ht"] = ht
                    emit(fthunks[0], "F0")
                    emit(G[0], "G0")
                    emit(fthunks[1], "F1")
                    _STAGE_MARKS.append((len(nc.inst_map), "D"))
                    mid["ps_z"] = zsum(mid["et"])
                    _STAGE_MARKS.append((len(nc.inst_map), "E"))
                    zb_norm(mid["et"], mid["ps_z"])
                    emit(G[1], "G1")
                    emit(fthunks[2], "F2")
                    emit(G[2], "G2")
                    for k in range(3, NF):
                        emit(fthunks[k], f"F{k}")
                else:
                    if pv is not None:
                        ht, fthunks = stage_f(pv["b"], pv["vc"], pv["et"])
                        pv["ht"] = ht
                    else:
                        fthunks = [None] * NF
                    emit(A[0], "A0")
                    emit(Cg[0], "C0")
                    emit(fthunks[0], "F0")
                    emit(G[0], "G0")
                    emit(A[1], "A1")
                    emit(Cg[1], "C1")
                    emit(fthunks[1], "F1")
                    emit(A[2], "A2")
                    emit(Cg[2], "C2")
                    emit(G[1], "G1")
                    emit(fthunks[2], "F2")
                    emit(Bg[0], "B0")
                    emit(Bg[1], "B1")
                    emit(fthunks[3], "F3")
                    if mid is not None:
                        _STAGE_MARKS.append((len(nc.inst_map), "D"))
                        mid["ps_z"] = zsum(mid["et"])
                        _STAGE_MARKS.append((len(nc.inst_map), "E"))
                        zb_norm(mid["et"], mid["ps_z"])
                    emit(Bg[2], "B2")
                    emit(Bg[3], "B3")
                    emit(fthunks[4], "F4")
                    emit(Bg[4], "B4")
                    emit(fthunks[5], "F5")
                    emit(Bg[5], "B5")
                    emit(Bg[6], "B6")
                    emit(Bg[7], "B7")
                    emit(G[2], "G2")

                # tail compression: the last sample's PV+tanh runs right
                # after its own normalize (one iteration early), and its
                # FC1+store rides after the second-to-last sample's.
                # F(s4) above plus the norm muls give PE/ACT work covering
                # the exp->zsum->recip->norm chain of s5.
                if mid is not None and mid["b"] == BPC - 1:
                    ht, fthunks = stage_f(mid["b"], mid["vc"], mid["et"])
                    mid["ht"] = ht
                    for k, th in enumerate(fthunks):
                        emit(th, f"F{k}")
                if old is not None and old["b"] == BPC - 2:
                    last = state[i - 2]
                    G2 = stage_g(last["b"], last["ht"], final=True)
                    for k, th in enumerate(G2):
                        emit(th, f"G{k}")
                    del state[i - 2]

                if old is not None:
                    del state[i - 3]
                if cur is not None:
                    state[i] = cur

    nc.finalize()
    return nc


def _get_nc():
    if "nc" not in _cache:
        _cache["nc"] = _build_program()
    return _cache["nc"]


def kernel(**inputs):
    from concourse.bass_utils import run_bass_kernel_spmd

    x = np.asarray(inputs["x"], dtype=np.float32)            # [B,S,H]
    mask = np.asarray(inputs["mask"], dtype=np.float32)      # [B,S]
    divide_pos = np.asarray(inputs["divide_pos"]).astype(np.int64)  # [B]
    Wq = np.asarray(inputs["Wq"], dtype=np.float32)
    bq = np.asarray(inputs["bq"], dtype=np.float32)
    Wk = np.asarray(inputs["Wk"], dtype=np.float32)
    bk = np.asarray(inputs["bk"], dtype=np.float32)
    Wv = np.asarray(inputs["Wv"], dtype=np.float32)
    bv = np.asarray(inputs["bv"], dtype=np.float32)
    W0 = np.asarray(inputs["W0"], dtype=np.float32)
    b0 = np.asarray(inputs["b0"], dtype=np.float32)
    W1 = np.asarray(inputs["W1"], dtype=np.float32)
    b1 = np.asarray(inputs["b1"], dtype=np.float32)

    bf = ml_dtypes.bfloat16
    f8 = ml_dtypes.float8_e4m3
    f85 = ml_dtypes.float8_e5m2

    # ---- host-side fusion + prep ----
    Bsc = (Wq.T @ Wk) * RS                # scores core: x @ Bsc @ x.T
    Wc = W0 @ Wv                          # fused V.W0
    b0p = (b0 + W0 @ bv).astype(np.float32)

    # majority-first permutation: positions [0,SH) hold SH tokens of the
    # sample's larger segment, so every (k<SH, q<SH) pair is same-segment.
    pos = np.arange(S)
    isq0 = (pos[None, :] < divide_pos[:, None])               # [B,S] bool
    perm = np.empty((B, S), np.int64)
    for bidx in range(B):
        maj = isq0[bidx] if divide_pos[bidx] >= SH else ~isq0[bidx]
        idx_maj = np.flatnonzero(maj)
        idx_min = np.flatnonzero(~maj)
        perm[bidx, :SH] = idx_maj[:SH]
        perm[bidx, SH:] = np.concatenate([idx_maj[SH:], idx_min])

    xp = np.take_along_axis(x, perm[:, :, None], axis=1)      # [B,S,H]
    maskp = np.take_along_axis(mask, perm, axis=1)            # [B,S]
    isq = np.take_along_axis(isq0, perm, axis=1).astype(np.float32)

    u = (xp @ (Wk.T @ bq)) * (RS * SB_OUT)    # [B,S] k-side bias term (x64)
    vq = (xp @ (Wq.T @ bk)) * (RS * SB_OUT)   # [B,S] q-side bias term (x64)
    c = float(bq @ bk) * RS * SB_OUT

    xT = np.ascontiguousarray(xp.transpose(0, 2, 1)).astype(f8)  # [B,H,S]
    Bsc8 = np.ascontiguousarray(Bsc * WS_B).astype(f8)           # layout [h, o]
    WcT8 = np.ascontiguousarray(Wc.T * WS_C).astype(f8)
    W1T8 = np.ascontiguousarray(W1.T * WS_1).astype(f8)

    # rank-4 mask/bias factors per sample (all x64 to match P1T scaling)
    rowQ = np.where(isq > 0, NEG, np.clip(maskp * SB_OUT, NEG, None))  # [B,S]
    rowC = np.where(isq > 0, 0.0, NEG)                                 # [B,S]
    ones = np.ones((B, S), np.float32)
    # rows r=2b+a laid out [a(partition), b(free pair)]: DR contracts (a,b)
    l4 = np.stack([rowQ, rowC, u + c, ones], axis=1).astype(f85)      # [B,4,S]
    r4 = np.stack([isq, 1.0 - isq, ones, vq], axis=1).astype(f85)     # [B,4,S]
    l4 = l4.reshape(B, 2, 2, S).transpose(2, 0, 1, 3).copy()          # [2,B,2,S]
    r4 = r4.reshape(B, 2, 2, S).transpose(2, 0, 1, 3).copy()          # [2,B,2,S]

    nc = _get_nc()
    in_maps = []
    for cid in range(NCORES):
        sl = slice(cid * BPC, (cid + 1) * BPC)
        in_maps.append({
            "xT": xT[sl],
            "Bsc": Bsc8, "WcT": WcT8, "W1T": W1T8, "b0p": b0p,
            "l4": l4[:, sl], "r4": r4[:, sl],
        })

    res = run_bass_kernel_spmd(nc, in_maps, core_ids=list(range(NCORES)))
    outT = np.concatenate(
        [np.asarray(r["outT"], dtype=np.float32) for r in res.results], axis=0
    )  # [B,H,S] (permuted along S)
    outp = outT.transpose(0, 2, 1)                            # [B,S,H]
    out = np.empty_like(outp)
    np.put_along_axis(out, perm[:, :, None], outp, axis=1)    # un-permute
    out = out * np.float32(1.0 / WS_1) + b1 + x
    return out.astype(np.float32)
